# revision 46
# baseline (speedup 1.0000x reference)
"""8-core Trainium2 Bass kernel for nn_DeepSeekClone (moe_routing).

Layout: activations are REPLICATED feature-major on every core as 16 (128,S)
"yT" tiles (features on partitions, tokens free), so every matmul consumes
weights (Din,Dout) as stationary and x^T as moving with zero activation
transposes after the embedding.

Sharding:
  - Attention: head-parallel (2 of 16 heads per core).
  - MoE: expert-parallel (expert c on core c), dense-per-expert with top-2
    masked gate weights (matches reference numerics exactly).
  - FFN: Megatron column/row split of ffn1/ffn2.
  - Combines: ONE AllReduce per token-half per combine (4/layer).  The
    residual (ln1-scaled x, divided by 8 so the 8-way sum reconstructs it),
    the attention rows (one-hot masked to the owning core) and biases are
    injected into the AR inputs, so no separate AllGather is ever needed —
    each AR directly yields the full next activation on every core.
  - Final projection: vocab-split (4000 cols/core); host concatenates.

Precision: top-2 routing is discrete — the reference's expert choices must be
reproduced exactly.  Layer-0's gate reads the exact embedding, so its stats
and gate matmuls run in true fp32; all other matmuls (including the rest of
layer 0) run in float32r (fp32 data at ~bf16 matmul speed).  Layer-1's gate
matmul is fp32 on the (slightly f32r-perturbed) layer-1 input; measured gate
margins (min top2-vs-top3 prob gap ~1e-4) dwarf the ~1e-5 perturbation.
"""
import sys

sys.path.insert(0, "/opt/trn_rl_repo")

import numpy as np

import concourse.bass as bass
import concourse.mybir as mybir
import concourse.tile as tile
from concourse import bacc
from concourse.bass_utils import run_bass_kernel_spmd
from concourse.masks import make_identity

V, D, L, E, H, TOPK, S, B = 32000, 2048, 2, 8, 16, 2, 1024, 1
DH = D // H          # 128
DFF = 4 * D          # 8192
EPS = 1e-6
NC = 8
P = 128
FS = D // NC         # 256 features/core (embedding slice)
FT = FS // P         # 2
HPC = H // NC        # 2 heads/core
DFS = DFF // NC      # 1024 ffn cols/core
UT = DFS // P        # 8
VS = V // NC         # 4000 vocab cols/core
KT = D // P          # 16
ST = S // P          # 8
NJ = S // 512        # 2 token halves
HS = S // NJ         # 512
F32 = mybir.dt.float32
F32R = mybir.dt.float32r
I32 = mybir.dt.int32
AX = mybir.AxisListType.X
AF = mybir.ActivationFunctionType
ALU = mybir.AluOpType
RG = [list(range(NC))]

_CACHE = {}


def _f(x):
    return np.ascontiguousarray(np.asarray(x), dtype=np.float32)


def _bcol(vec, nt):
    """(nt*128,) vector -> (128, nt) with [p, i] = vec[i*128 + p]."""
    return np.ascontiguousarray(_f(vec).reshape(nt, P).T)


def prepare_in_maps(inputs):
    tokens = np.ascontiguousarray(np.asarray(inputs["tokens"]).reshape(S), dtype=np.int32)
    embed = _f(inputs["embed"])
    qkv_w = _f(inputs["qkv_w"]); qkv_b = _f(inputs["qkv_b"])
    gate_w = _f(inputs["gate_w"]); gate_b = _f(inputs["gate_b"])
    exp_w = _f(inputs["exp_w"]); exp_b = _f(inputs["exp_b"])
    ln1_s = _f(inputs["ln1_s"]); ln1_b = _f(inputs["ln1_b"])
    ln2_s = _f(inputs["ln2_s"]); ln2_b = _f(inputs["ln2_b"])
    ffn1_w = _f(inputs["ffn1_w"]); ffn1_b = _f(inputs["ffn1_b"])
    ffn2_w = _f(inputs["ffn2_w"]); ffn2_b = _f(inputs["ffn2_b"])
    out_w = _f(inputs["out_w"]); out_b = _f(inputs["out_b"])

    tok8 = np.ascontiguousarray(tokens.reshape(ST, P))
    maps = []
    for c in range(NC):
        fidx = np.concatenate([np.arange(c * P, (c + 1) * P),
                               np.arange((NC + c) * P, (NC + c + 1) * P)])
        m = {"tok": tok8, "emb": np.ascontiguousarray(embed[:, fidx])}
        ohot = np.zeros((P, KT), np.float32)
        ohot[:, c] = 1.0
        ohot[:, NC + c] = 1.0
        m["ohot"] = ohot
        for l in range(L):
            hs = [c, NC + c]
            s1 = ln1_s[l][:, None]; b1v = ln1_b[l]
            s2 = ln2_s[l][:, None]; b2v = ln2_b[l]
            wqk = np.concatenate(
                [qkv_w[l][:, 0 * D + h * DH:0 * D + (h + 1) * DH] for h in hs]
                + [qkv_w[l][:, 1 * D + h * DH:1 * D + (h + 1) * DH] for h in hs], axis=1)
            bqk = np.concatenate(
                [qkv_b[l][0 * D + h * DH:0 * D + (h + 1) * DH] for h in hs]
                + [qkv_b[l][1 * D + h * DH:1 * D + (h + 1) * DH] for h in hs])
            wv = np.concatenate(
                [qkv_w[l][:, 2 * D + h * DH:2 * D + (h + 1) * DH] for h in hs], axis=1)
            bv = np.concatenate(
                [qkv_b[l][2 * D + h * DH:2 * D + (h + 1) * DH] for h in hs])
            # fold LN1 scale/bias into qkv/v/expert weights, LN2 into ffn1
            bqk = bqk + b1v @ wqk
            wqk = s1 * wqk
            bv = bv + b1v @ wv
            wv = s1 * wv
            we_c = s1 * exp_w[l][c]
            be_c = exp_b[l][c] + b1v @ exp_w[l][c]
            w1_c = s2 * ffn1_w[l][:, c * DFS:(c + 1) * DFS]
            b1_c = ffn1_b[l][c * DFS:(c + 1) * DFS] + b2v @ ffn1_w[l][:, c * DFS:(c + 1) * DFS]
            sel = np.zeros((E, P), np.float32); sel[c, :] = 1.0
            # gate from unnormalized x: logits = rinv*(x@ws - mu*ce) + de
            ws1 = ln1_s[l][:, None] * gate_w[l]                      # (D, E)
            ce1 = ws1.sum(axis=0).reshape(E, 1)
            de1 = (ln1_b[l] @ gate_w[l] + gate_b[l]).reshape(E, 1)
            m.update({
                f"wqk_{l}": np.ascontiguousarray(wqk),
                f"bqk_{l}": _bcol(bqk, 2 * HPC),
                f"wv_{l}": np.ascontiguousarray(wv),
                f"bv_{l}": np.ascontiguousarray(bv.reshape(1, HPC * DH)),
                f"ws_{l}": np.ascontiguousarray(ws1),
                f"ce_{l}": np.ascontiguousarray(ce1),
                f"de_{l}": np.ascontiguousarray(de1),
                f"sel_{l}": sel,
                f"we_{l}": np.ascontiguousarray(we_c),
                f"be_{l}": _bcol(be_c, KT),
                # residual injection: y = s1*yTn + b1, divided by 8 so the
                # 8-way AllReduce sum reconstructs it exactly once
                f"rs1_{l}": _bcol(ln1_s[l] / NC, KT),
                f"rb1_{l}": _bcol(ln1_b[l] / NC, KT),
                f"w1_{l}": np.ascontiguousarray(w1_c),
                f"b1_{l}": _bcol(b1_c, UT),
                f"w2_{l}": np.ascontiguousarray(ffn2_w[l][c * DFS:(c + 1) * DFS, :]),
                f"b2e_{l}": _bcol(ffn2_b[l] / NC, KT),
            })
        m["wo"] = np.ascontiguousarray(out_w[:, c * VS:(c + 1) * VS])
        m["bo"] = np.ascontiguousarray(out_b[c * VS:(c + 1) * VS].reshape(1, VS))
        maps.append(m)
    return maps


def build_nc():
    nc = bacc.Bacc("TRN2", target_bir_lowering=False, debug=False, num_devices=NC)

    tok = nc.dram_tensor("tok", [ST, P], I32, kind="ExternalInput")
    emb = nc.dram_tensor("emb", [V, FS], F32, kind="ExternalInput")
    ins = {"ohot": nc.dram_tensor("ohot", [P, KT], F32, kind="ExternalInput")}
    for l in range(L):
        for nm, shape, d in [
            (f"wqk_{l}", [D, 4 * DH], F32R), (f"bqk_{l}", [P, 4], F32),
            (f"wv_{l}", [D, 2 * DH], F32R), (f"bv_{l}", [1, 2 * DH], F32),
            (f"ws_{l}", [D, E], F32), (f"ce_{l}", [E, 1], F32),
            (f"de_{l}", [E, 1], F32), (f"sel_{l}", [E, P], F32),
            (f"we_{l}", [D, D], F32R), (f"be_{l}", [P, KT], F32),
            (f"rs1_{l}", [P, KT], F32), (f"rb1_{l}", [P, KT], F32),
            (f"w1_{l}", [D, DFS], F32R), (f"b1_{l}", [P, UT], F32),
            (f"w2_{l}", [DFS, D], F32R), (f"b2e_{l}", [P, KT], F32),
        ]:
            ins[nm] = nc.dram_tensor(nm, shape, d, kind="ExternalInput")
    wo = nc.dram_tensor("wo", [D, VS], F32R, kind="ExternalInput")
    bo = nc.dram_tensor("bo", [1, VS], F32, kind="ExternalInput")
    out = nc.dram_tensor("out", [S, VS], F32, kind="ExternalOutput")

    with tile.TileContext(nc) as tc:
        _build_body(nc, tc, tok, emb, ins, wo, bo, out)
    nc.compile()
    return nc


def _build_body(nc, tc, tok, emb, ins, wo, bo, out):
    from contextlib import ExitStack

    with ExitStack() as ctx:
        cb = ctx.enter_context(tc.tile_pool(name="cb", bufs=1))
        act = ctx.enter_context(tc.tile_pool(name="act", bufs=1))
        pp = ctx.enter_context(tc.tile_pool(name="pp", bufs=1, space="PSUM"))
        dr = ctx.enter_context(tc.tile_pool(name="dr", bufs=1, space="DRAM"))

        # ---------- constants ----------
        ident_f = cb.tile([P, P], F32, name="ident_f", tag="ident_f")
        make_identity(nc, ident_f)
        ident_r = cb.tile([P, P], F32R, name="ident_r", tag="ident_r")
        nc.vector.tensor_copy(ident_r[:], ident_f[:])
        ones_cf = cb.tile([P, 1], F32, name="ones_cf", tag="ones_cf")
        nc.vector.memset(ones_cf[:], 1.0)
        ones_cr = cb.tile([P, 1], F32R, name="ones_cr", tag="ones_cr")
        nc.vector.tensor_copy(ones_cr[:], ones_cf[:])
        ones_rf = cb.tile([1, P], F32, name="ones_rf", tag="ones_rf")
        nc.vector.tensor_copy(ones_rf[:], ones_cf[:1, :].to_broadcast([1, P]))
        ones_rr = cb.tile([1, P], F32R, name="ones_rr", tag="ones_rr")
        nc.vector.tensor_copy(ones_rr[:], ones_rf[:])
        eps_pp = cb.tile([P, 1], F32, name="eps_pp", tag="eps_pp")
        nc.vector.memset(eps_pp[:], EPS)
        ohot_pp = cb.tile([P, KT], F32, name="ohot_pp", tag="ohot_pp")
        nc.sync.dma_start(ohot_pp[:], ins["ohot"][:])

        # ---------- persistent activation tiles (all fp32 bits) ----------
        # yT: the replicated full activation, feature-major.
        yT = [act.tile([P, S], F32, name=f"yT{ki}", tag=f"yT{ki}") for ki in range(KT)]
        aoT = [act.tile([P, S], F32, name=f"aoT{h}", tag=f"aoT{h}") for h in range(HPC)]
        ew_b = act.tile([P, S], F32, name="ew_b", tag="ew_b")
        gT = act.tile([E, S], F32, name="gT", tag="gT")

        # final-projection bias, f32r: folded into the final matmul as an
        # extra contraction row (ones stationary row x bias moving)
        bo_r = cb.tile([1, VS], F32R, name="bo_r", tag="bo_r")
        nc.gpsimd.dma_start(bo_r[:], bo[:])

        def yr(ki, c0, c1):
            return yT[ki][:, c0:c1].bitcast(F32R)

        # =============== embedding gather -> x_cat (own slice) ===============
        # x_cat holds both 128-feature blocks side by side: cols [0,S) are
        # feature block c, cols [S,2S) block 8+c -> ONE AllGather suffices.
        def embed_gather(x_cat):
            with tc.tile_pool(name="embp", bufs=1) as wp:
                tok_t = wp.tile([P, ST], I32, name="tok_t", tag="tok_t")
                nc.sync.dma_start(tok_t[:], tok[:, :].rearrange("c p -> p c"))
                gts = []
                for cbk in range(ST):
                    g = wp.tile([P, FS], F32, name="g", tag=f"g{cbk}")
                    nc.gpsimd.indirect_dma_start(
                        out=g[:], out_offset=None, in_=emb[:],
                        in_offset=bass.IndirectOffsetOnAxis(ap=tok_t[:, cbk:cbk + 1], axis=0))
                    gts.append(g)
                for fi in range(FT):
                    for cbk in range(ST):
                        tp = pp.tile([P, P], F32, name="tp", tag=f"b{(cbk * FT + fi) % 4}")
                        nc.tensor.transpose(tp[:], gts[cbk][:, fi * P:(fi + 1) * P],
                                            ident_f[:])
                        dst = x_cat[:, fi * S + cbk * P:fi * S + (cbk + 1) * P]
                        if (cbk + fi) % 2 == 0:
                            nc.vector.tensor_copy(dst, tp[:])
                        else:
                            nc.scalar.copy(dst, tp[:])

        # =============== LN1 of layer 0: gather-first, fp32 stats ===========
        # AllGather the unnormalized x0 slices, accumulate full fp32 stats +
        # fp32 gate projection per arriving tile, then normalize in place.
        def ln_gather(l, x_cat):
            with tc.tile_pool(name=f"lng_{l}", bufs=1) as wp:
                ws_t = wp.tile([P, KT * E], F32, name="ws_t", tag="ws_t")
                nc.sync.dma_start(ws_t[:].rearrange("p (kt e) -> p kt e", e=E),
                                  ins[f"ws_{l}"][:, :].rearrange("(kt p) e -> p kt e", p=P))
                ag_in = dr.tile([P, 2 * S], F32, name="agin", tag="agin")
                ag_out = dr.tile([NC * P, 2 * S], F32, name="agout", tag="agout",
                                 addr_space="Shared")
                nc.sync.dma_start(ag_in[:], x_cat[:])
                nc.gpsimd.collective_compute("AllGather", ALU.bypass, replica_groups=RG,
                                             ins=[ag_in.opt()], outs=[ag_out.opt()])
                ps_st = [[pp.tile([1, HS], F32, name="ps_st", tag=f"b{st * NJ + nj}")
                          for nj in range(NJ)] for st in range(2)]
                ps_g = [pp.tile([E, HS], F32, name="ps_g", tag=f"b{4 + nj}")
                        for nj in range(NJ)]
                for ki in range(KT):
                    h, r = ki // ST, ki % ST
                    nc.sync.dma_start(yT[ki][:],
                                      ag_out[r * P:(r + 1) * P, h * S:(h + 1) * S])
                    sq = wp.tile([P, S], F32, name="sq", tag="sq", bufs=3)
                    nc.scalar.activation(sq[:], yT[ki][:], AF.Square)
                    for nj in range(NJ):
                        nc.tensor.matmul(ps_st[0][nj][:], ones_cf[:],
                                         yT[ki][:, nj * HS:(nj + 1) * HS],
                                         start=(ki == 0), stop=(ki == KT - 1))
                        nc.tensor.matmul(ps_st[1][nj][:], ones_cf[:],
                                         sq[:, nj * HS:(nj + 1) * HS],
                                         start=(ki == 0), stop=(ki == KT - 1))
                        nc.tensor.matmul(ps_g[nj][:], ws_t[:, ki * E:(ki + 1) * E],
                                         yT[ki][:, nj * HS:(nj + 1) * HS],
                                         start=(ki == 0), stop=(ki == KT - 1))
                mu_row = wp.tile([1, S], F32, name="mu_row", tag="mu_row")
                e2_row = wp.tile([1, S], F32, name="e2_row", tag="e2_row")
                for nj in range(NJ):
                    nc.scalar.mul(mu_row[:, nj * HS:(nj + 1) * HS], ps_st[0][nj][:], 1.0 / D)
                    nc.scalar.mul(e2_row[:, nj * HS:(nj + 1) * HS], ps_st[1][nj][:], 1.0 / D)
                var_row = wp.tile([1, S], F32, name="var_row", tag="var_row")
                nc.vector.tensor_mul(var_row[:], mu_row[:], mu_row[:])
                nc.vector.tensor_sub(var_row[:], e2_row[:], var_row[:])
                sd_row = wp.tile([1, S], F32, name="sd_row", tag="sd_row")
                nc.scalar.activation(sd_row[:], var_row[:], AF.Sqrt, bias=eps_pp[:1, :])
                rinv_row = wp.tile([1, S], F32, name="rinv_row", tag="rinv_row")
                nc.vector.reciprocal(rinv_row[:], sd_row[:])
                # gate logits gT = rinv*(graw - mu*ce) + de  (fp32)
                ce_pp = wp.tile([E, 1], F32, name="ce_pp", tag="ce_pp")
                nc.sync.dma_start(ce_pp[:], ins[f"ce_{l}"][:])
                de_pp = wp.tile([E, 1], F32, name="de_pp", tag="de_pp")
                nc.sync.dma_start(de_pp[:], ins[f"de_{l}"][:])
                graw = wp.tile([E, S], F32, name="graw", tag="graw")
                mu8 = wp.tile([E, S], F32, name="mu8", tag="mu8")
                rinv8 = wp.tile([E, S], F32, name="rinv8", tag="rinv8")
                for nj in range(NJ):
                    nc.scalar.copy(graw[:, nj * HS:(nj + 1) * HS], ps_g[nj][:])
                    ps8 = pp.tile([E, HS], F32, name="ps8", tag=f"b{4 + nj}")
                    nc.tensor.matmul(ps8[:], ones_rf[:1, :E],
                                     mu_row[:, nj * HS:(nj + 1) * HS],
                                     start=True, stop=True)
                    nc.scalar.copy(mu8[:, nj * HS:(nj + 1) * HS], ps8[:])
                    ps8b = pp.tile([E, HS], F32, name="ps8b", tag=f"b{4 + nj}")
                    nc.tensor.matmul(ps8b[:], ones_rf[:1, :E],
                                     rinv_row[:, nj * HS:(nj + 1) * HS],
                                     start=True, stop=True)
                    nc.scalar.copy(rinv8[:, nj * HS:(nj + 1) * HS], ps8b[:])
                t1g = wp.tile([E, S], F32, name="t1g", tag="t1g")
                nc.vector.scalar_tensor_tensor(t1g[:], mu8[:], ce_pp[:, 0:1], graw[:],
                                               ALU.mult, ALU.subtract)
                nc.vector.tensor_mul(t1g[:], t1g[:], rinv8[:])
                nc.vector.tensor_scalar(gT[:], t1g[:], -1.0, de_pp[:, 0:1],
                                        ALU.mult, ALU.add)
                # broadcast mu/rinv across partitions (f32r-rounded rows) + normalize
                mu_row_r = wp.tile([1, S], F32R, name="mu_row_r", tag="mu_row_r")
                nc.scalar.copy(mu_row_r[:], mu_row[:])
                rinv_row_r = wp.tile([1, S], F32R, name="rinv_row_r", tag="rinv_row_r")
                nc.scalar.copy(rinv_row_r[:], rinv_row[:])
                mu_b = wp.tile([P, S], F32, name="mu_b", tag="mu_b")
                rinv_b = wp.tile([P, S], F32, name="rinv_b", tag="rinv_b")
                for src_row, dst in ((mu_row_r, mu_b), (rinv_row_r, rinv_b)):
                    for nj in range(NJ):
                        psb = pp.tile([P, HS], F32, name="psb", tag=f"b{6 + nj}")
                        nc.tensor.matmul(psb[:], ones_rr[:],
                                         src_row[:, nj * HS:(nj + 1) * HS],
                                         start=True, stop=True)
                        nc.scalar.copy(dst[:, nj * HS:(nj + 1) * HS], psb[:])
                for ki in range(KT):
                    w_ap = yT[ki][:].bitcast(F32R)
                    nc.vector.tensor_sub(w_ap, yT[ki][:], mu_b[:])
                    nc.vector.tensor_mul(w_ap, yT[ki][:], rinv_b[:])

        # =============== per-token-half LN from an AR output ===============
        # src[nj] are (D, HS) Shared DRAM tiles holding the full summed
        # activation.  Loads into yT[:, half], computes f32r stats (+ optional
        # fp32 gate logits) and normalizes in place (skipped for the final
        # projection input, which is consumed raw).
        def ln_local(l, name, src, nj, with_gate, normalize=True, ws_t=None):
            c0, c1 = nj * HS, (nj + 1) * HS
            if not normalize and not with_gate:
                # raw load only (final-projection input); F32R-declared dest so
                # the f32r final matmuls accept it (gpsimd DMA may cast)
                for ki in range(KT):
                    nc.gpsimd.dma_start(yT[ki][:, c0:c1].bitcast(F32R),
                                        src[ki * P:(ki + 1) * P, :])
                return
            base = nj * 4
            with tc.tile_pool(name=f"lnl_{name}_{nj}", bufs=1) as wp:
                ps_s = pp.tile([1, HS], F32, name="ps_s", tag=f"b{base}")
                ps_q = pp.tile([1, HS], F32, name="ps_q", tag=f"b{base + 1}")
                if with_gate:
                    ps_g = pp.tile([E, HS], F32, name="ps_g", tag=f"b{base + 2}")
                for ki in range(KT):
                    nc.gpsimd.dma_start(yT[ki][:, c0:c1].bitcast(F32R),
                                        src[ki * P:(ki + 1) * P, :])
                    sq = wp.tile([P, HS], F32R, name="sq", tag="sq", bufs=2)
                    nc.scalar.activation(sq[:], yT[ki][:, c0:c1], AF.Square)
                    nc.tensor.matmul(ps_s[:], ones_cr[:], yr(ki, c0, c1),
                                     start=(ki == 0), stop=(ki == KT - 1))
                    nc.tensor.matmul(ps_q[:], ones_cr[:], sq[:],
                                     start=(ki == 0), stop=(ki == KT - 1))
                    if with_gate:
                        nc.tensor.matmul(ps_g[:], ws_t[:, ki * E:(ki + 1) * E],
                                         yT[ki][:, c0:c1],
                                         start=(ki == 0), stop=(ki == KT - 1))
                mu_row = wp.tile([1, HS], F32, name="mu_row", tag="mu_row")
                nc.scalar.mul(mu_row[:], ps_s[:], 1.0 / D)
                e2_row = wp.tile([1, HS], F32, name="e2_row", tag="e2_row")
                nc.scalar.mul(e2_row[:], ps_q[:], 1.0 / D)
                var_row = wp.tile([1, HS], F32, name="var_row", tag="var_row")
                nc.vector.tensor_mul(var_row[:], mu_row[:], mu_row[:])
                nc.vector.tensor_sub(var_row[:], e2_row[:], var_row[:])
                sd_row = wp.tile([1, HS], F32, name="sd_row", tag="sd_row")
                nc.scalar.activation(sd_row[:], var_row[:], AF.Sqrt, bias=eps_pp[:1, :])
                rinv_row = wp.tile([1, HS], F32, name="rinv_row", tag="rinv_row")
                nc.vector.reciprocal(rinv_row[:], sd_row[:])
                if with_gate:
                    ce_pp = wp.tile([E, 1], F32, name="ce_pp", tag="ce_pp")
                    nc.sync.dma_start(ce_pp[:], ins[f"ce_{l}"][:])
                    de_pp = wp.tile([E, 1], F32, name="de_pp", tag="de_pp")
                    nc.sync.dma_start(de_pp[:], ins[f"de_{l}"][:])
                    graw = wp.tile([E, HS], F32, name="graw", tag="graw")
                    nc.scalar.copy(graw[:], ps_g[:])
                    ps8 = pp.tile([E, HS], F32, name="ps8", tag=f"b{base + 2}")
                    nc.tensor.matmul(ps8[:], ones_rf[:1, :E], mu_row[:],
                                     start=True, stop=True)
                    mu8 = wp.tile([E, HS], F32, name="mu8", tag="mu8")
                    nc.scalar.copy(mu8[:], ps8[:])
                    ps8b = pp.tile([E, HS], F32, name="ps8b", tag=f"b{base + 2}")
                    nc.tensor.matmul(ps8b[:], ones_rf[:1, :E], rinv_row[:],
                                     start=True, stop=True)
                    rinv8 = wp.tile([E, HS], F32, name="rinv8", tag="rinv8")
                    nc.scalar.copy(rinv8[:], ps8b[:])
                    t1g = wp.tile([E, HS], F32, name="t1g", tag="t1g")
                    nc.vector.scalar_tensor_tensor(t1g[:], mu8[:], ce_pp[:, 0:1], graw[:],
                                                   ALU.mult, ALU.subtract)
                    nc.vector.tensor_mul(t1g[:], t1g[:], rinv8[:])
                    nc.vector.tensor_scalar(gT[:, c0:c1], t1g[:], -1.0, de_pp[:, 0:1],
                                            ALU.mult, ALU.add)
                if normalize:
                    mu_row_r = wp.tile([1, HS], F32R, name="mu_row_r", tag="mu_row_r")
                    nc.scalar.copy(mu_row_r[:], mu_row[:])
                    rinv_row_r = wp.tile([1, HS], F32R, name="rinv_row_r", tag="rinv_row_r")
                    nc.scalar.copy(rinv_row_r[:], rinv_row[:])
                    mu_b = wp.tile([P, HS], F32, name="mu_b", tag="mu_b")
                    rinv_b = wp.tile([P, HS], F32, name="rinv_b", tag="rinv_b")
                    for src_row, dst, bt in ((mu_row_r, mu_b, base), (rinv_row_r, rinv_b, base + 1)):
                        psb = pp.tile([P, HS], F32, name="psb", tag=f"b{bt}")
                        nc.tensor.matmul(psb[:], ones_rr[:], src_row[:],
                                         start=True, stop=True)
                        nc.scalar.copy(dst[:], psb[:])
                    for ki in range(KT):
                        w_ap = yr(ki, c0, c1)
                        nc.vector.tensor_sub(w_ap, yT[ki][:, c0:c1], mu_b[:])
                        nc.vector.tensor_mul(w_ap, yT[ki][:, c0:c1], rinv_b[:])

        # ---------- persistent attention tiles (reused across layers) ----------
        NHM = 2 * HPC
        qkT = [act.tile([P, S], F32R, name=f"qkT{mi}", tag=f"qkT{mi}") for mi in range(NHM)]
        v_sb = [act.tile([P, 2 * DH], F32R, name=f"v_sb{mi}", tag=f"v_sb{mi}")
                for mi in range(ST)]
        bqk_pp = act.tile([P, NHM], F32, name="bqk_pp", tag="bqk_pp")
        bv_row = act.tile([1, 2 * DH], F32, name="bv_row", tag="bv_row")
        bv_b = act.tile([P, 2 * DH], F32, name="bv_b", tag="bv_b")
        ws_tl = act.tile([P, KT * E], F32, name="ws_tl", tag="ws_tl")

        def attn_prep(l):
            nc.sync.dma_start(bqk_pp[:], ins[f"bqk_{l}"][:])
            nc.sync.dma_start(bv_row[:], ins[f"bv_{l}"][:])
            pbv = pp.tile([P, 2 * DH], F32, name="pbv", tag="b3")
            nc.tensor.matmul(pbv[:], ones_rf[:], bv_row[:], start=True, stop=True)
            nc.scalar.copy(bv_b[:], pbv[:])

        def attn_qkv_half(l, nj):
            with tc.tile_pool(name=f"qkv_{l}_{nj}", bufs=1) as wp:
                psq = [pp.tile([P, HS], F32, name=f"psq{mi}", tag=f"b{nj * 4 + mi}")
                       for mi in range(NHM)]
                for ki in range(KT):
                    wk = wp.tile([P, 4 * DH], F32R, name="wqk_t", tag="wqk_t", bufs=6)
                    nc.sync.dma_start(wk[:], ins[f"wqk_{l}"][ki * P:(ki + 1) * P, :])
                    for mi in range(NHM):
                        nc.tensor.matmul(psq[mi][:], wk[:, mi * P:(mi + 1) * P],
                                         yr(ki, nj * HS, (nj + 1) * HS),
                                         start=(ki == 0), stop=(ki == KT - 1))
                for mi in range(NHM):
                    nc.scalar.activation(qkT[mi][:, nj * HS:(nj + 1) * HS], psq[mi][:],
                                         AF.Identity, bias=bqk_pp[:, mi:mi + 1])

        def attn_v_half(l, vg):
            with tc.tile_pool(name=f"vph_{l}_{vg}", bufs=1) as wp:
                psv = [pp.tile([P, 2 * DH], F32, name=f"psv{mi}", tag=f"b{vg * 4 + mi}")
                       for mi in range(4)]
                for ki in range(KT):
                    wk = wp.tile([P, 2 * DH], F32R, name="wv_t", tag="wv_t", bufs=6)
                    nc.sync.dma_start(wk[:], ins[f"wv_{l}"][ki * P:(ki + 1) * P, :])
                    for mi in range(4):
                        tb = vg * 4 + mi
                        nc.tensor.matmul(psv[mi][:], yr(ki, tb * P, (tb + 1) * P), wk[:],
                                         start=(ki == 0), stop=(ki == KT - 1))
                for mi in range(4):
                    nc.vector.tensor_add(v_sb[vg * 4 + mi][:], psv[mi][:], bv_b[:])

        def attn_core(l):
            with tc.tile_pool(name=f"attc_{l}", bufs=1) as wp:
                for h in range(HPC):
                    qh, kh = qkT[h], qkT[HPC + h]
                    # scores computed pre-transposed: stationary = key block,
                    # moving = queries -> AT[k, q] tiles straight off the PE.
                    # exp without max-subtraction is safe: |scores/sqrt(dh)|
                    # stays a few units for this model, far from fp32 overflow.
                    AT = [wp.tile([P, S], F32R, name=f"AT{kb}", tag=f"AT{kb}")
                          for kb in range(ST)]
                    for kb in range(ST):
                        for nj in range(NJ):
                            pss = pp.tile([P, HS], F32, name="pss",
                                          tag=f"b{(kb % 2) * 2 + nj}")
                            nc.tensor.matmul(pss[:], kh[:, kb * P:(kb + 1) * P],
                                             qh[:, nj * HS:(nj + 1) * HS],
                                             start=True, stop=True)
                            nc.scalar.activation(AT[kb][:, nj * HS:(nj + 1) * HS], pss[:],
                                                 AF.Exp, scale=1.0 / float(np.sqrt(DH)))
                    ao = aoT[h]
                    for nj in range(NJ):
                        po = pp.tile([P, HS], F32, name="po", tag=f"b{6 + nj}")
                        for kb in range(ST):
                            nc.tensor.matmul(po[:], v_sb[kb][:, h * DH:(h + 1) * DH],
                                             AT[kb][:, nj * HS:(nj + 1) * HS],
                                             start=(kb == 0), stop=(kb == ST - 1))
                        nc.vector.tensor_copy(ao[:, nj * HS:(nj + 1) * HS], po[:])
                    # per-token softmax denominator: column-sums of A^T
                    srow = wp.tile([1, S], F32, name="srow_a", tag="srow_a")
                    for nj in range(NJ):
                        ps_sr = pp.tile([1, HS], F32, name="ps_sr", tag="b4")
                        for kb in range(ST):
                            nc.tensor.matmul(ps_sr[:], ones_cr[:],
                                             AT[kb][:, nj * HS:(nj + 1) * HS],
                                             start=(kb == 0), stop=(kb == ST - 1))
                        nc.scalar.copy(srow[:, nj * HS:(nj + 1) * HS], ps_sr[:])
                    rrow = wp.tile([1, S], F32, name="rrow", tag="rrow")
                    nc.vector.reciprocal(rrow[:], srow[:])
                    inv_b = wp.tile([P, S], F32, name="inv_b", tag="inv_b")
                    for nj in range(NJ):
                        pbc = pp.tile([P, HS], F32, name="pbc", tag="b5")
                        nc.tensor.matmul(pbc[:], ones_rf[:],
                                         rrow[:, nj * HS:(nj + 1) * HS],
                                         start=True, stop=True)
                        nc.scalar.copy(inv_b[:, nj * HS:(nj + 1) * HS], pbc[:])
                    nc.vector.tensor_mul(ao[:], ao[:], inv_b[:])

        # ---------- gate top-2 selection (gT computed in ln) ----------
        def gate_block(l):
            with tc.tile_pool(name=f"gate_{l}", bufs=1) as wp:
                sel = wp.tile([E, P], F32, name="sel", tag="sel")
                nc.sync.dma_start(sel[:], ins[f"sel_{l}"][:])
                ewT = wp.tile([E, S], F32, name="ewT", tag="ewT")
                for qb in range(ST):
                    tpg = pp.tile([P, E], F32, name="tpg", tag="b6")
                    nc.tensor.transpose(tpg[:], gT[:, qb * P:(qb + 1) * P], ident_f[:E, :E])
                    gtok = wp.tile([P, E], F32, name="gtok", tag="gtok", bufs=2)
                    nc.vector.tensor_copy(gtok[:], tpg[:])
                    nm1 = wp.tile([P, 1], F32, name="nm1", tag="nm1", bufs=2)
                    nc.vector.reduce_max(nm1[:], gtok[:], axis=AX, negate=True)
                    eg = wp.tile([P, E], F32, name="eg", tag="eg", bufs=2)
                    gs = wp.tile([P, 1], F32, name="gs", tag="gs", bufs=2)
                    nc.scalar.activation(eg[:], gtok[:], AF.Exp, bias=nm1[:], accum_out=gs[:])
                    rg_ = wp.tile([P, 1], F32, name="rg_", tag="rg_", bufs=2)
                    nc.vector.reciprocal(rg_[:], gs[:])
                    p_t = wp.tile([P, E], F32, name="p_t", tag="p_t", bufs=2)
                    nc.vector.tensor_scalar_mul(p_t[:], eg[:], rg_[:])
                    m1 = wp.tile([P, 1], F32, name="m1", tag="m1", bufs=2)
                    nc.vector.reduce_max(m1[:], p_t[:], axis=AX)
                    mask = wp.tile([P, E], F32, name="mask", tag="mask", bufs=2)
                    nc.vector.tensor_scalar(mask[:], p_t[:], m1[:], None, ALU.is_equal)
                    pmask = wp.tile([P, E], F32, name="pmask", tag="pmask", bufs=2)
                    nc.vector.scalar_tensor_tensor(pmask[:], mask[:], -1e30, p_t[:],
                                                   ALU.mult, ALU.add)
                    m2 = wp.tile([P, 1], F32, name="m2", tag="m2", bufs=2)
                    nc.vector.reduce_max(m2[:], pmask[:], axis=AX)
                    gemask = wp.tile([P, E], F32, name="gemask", tag="gemask", bufs=2)
                    nc.vector.tensor_scalar(gemask[:], p_t[:], m2[:], None, ALU.is_ge)
                    ew_t = wp.tile([P, E], F32, name="ew_t", tag="ew_t", bufs=2)
                    nc.vector.tensor_mul(ew_t[:], p_t[:], gemask[:])
                    tpe = pp.tile([E, P], F32, name="tpe", tag="b7")
                    nc.tensor.transpose(tpe[:], ew_t[:], ident_f[:])
                    nc.vector.tensor_copy(ewT[:, qb * P:(qb + 1) * P], tpe[:])
                for nj in range(NJ):
                    pe_ = pp.tile([P, HS], F32, name="pe_", tag="b6")
                    nc.tensor.matmul(pe_[:], sel[:], ewT[:, nj * HS:(nj + 1) * HS],
                                     start=True, stop=True)
                    nc.scalar.copy(ew_b[:, nj * HS:(nj + 1) * HS], pe_[:])

        # ---------- MoE partials + residual/attn injection + AR ----------
        def moe_phase(l):
            arm_in = [dr.tile([D, HS], F32, name=f"armi{nj}", tag=f"armi{nj}_{l}")
                      for nj in range(NJ)]
            arm_out = [dr.tile([D, HS], F32, name=f"armo{nj}", tag=f"armo{nj}_{l}",
                               addr_space="Shared") for nj in range(NJ)]
            with tc.tile_pool(name=f"moe_{l}", bufs=1) as wp:
                be_pp = wp.tile([P, KT], F32, name="be_pp", tag="be_pp")
                nc.sync.dma_start(be_pp[:], ins[f"be_{l}"][:])
                rs1_pp = wp.tile([P, KT], F32, name="rs1_pp", tag="rs1_pp")
                nc.sync.dma_start(rs1_pp[:], ins[f"rs1_{l}"][:])
                rb1_pp = wp.tile([P, KT], F32, name="rb1_pp", tag="rb1_pp")
                nc.sync.dma_start(rb1_pp[:], ins[f"rb1_{l}"][:])
                we_v = ins[f"we_{l}"]
                for nj in range(NJ):
                    c0, c1 = nj * HS, (nj + 1) * HS
                    for hg in range(4):
                        # nj0 alternates b0-3/b4-7; nj1 stays in its b4-7 class
                        # (b0-3 is ln2(nj0)+ffn1(nj0)'s while AR-m1 is in flight)
                        bs = (hg % 2) * 4 if nj == 0 else 4
                        psz = [pp.tile([P, HS], F32, name=f"psz{mi}", tag=f"b{bs + mi}")
                               for mi in range(4)]
                        for ki in range(KT):
                            wk = wp.tile([P, HS], F32R, name="we_t", tag="we_t", bufs=8)
                            nc.sync.dma_start(wk[:], we_v[ki * P:(ki + 1) * P,
                                                         hg * HS:(hg + 1) * HS])
                            for mi in range(4):
                                nc.tensor.matmul(psz[mi][:], wk[:, mi * P:(mi + 1) * P],
                                                 yr(ki, c0, c1),
                                                 start=(ki == 0), stop=(ki == KT - 1))
                        for mi in range(4):
                            km = hg * 4 + mi
                            z = wp.tile([P, HS], F32, name="z", tag="z", bufs=4)
                            nc.vector.scalar_tensor_tensor(z[:], psz[mi][:],
                                                           be_pp[:, km:km + 1],
                                                           ew_b[:, c0:c1],
                                                           ALU.add, ALU.mult)
                            rt = wp.tile([P, HS], F32, name="rt", tag="rt", bufs=4)
                            nc.scalar.activation(rt[:], yT[km][:, c0:c1], AF.Identity,
                                                 bias=rb1_pp[:, km:km + 1],
                                                 scale=rs1_pp[:, km:km + 1])
                            z2 = wp.tile([P, HS], F32, name="z2", tag="z2", bufs=4)
                            nc.vector.scalar_tensor_tensor(z2[:], aoT[km // ST][:, c0:c1],
                                                           ohot_pp[:, km:km + 1], rt[:],
                                                           ALU.mult, ALU.add)
                            zf = wp.tile([P, HS], F32, name="zf", tag="zf", bufs=4)
                            nc.vector.tensor_add(zf[:], z[:], z2[:])
                            nc.gpsimd.dma_start(arm_in[nj][km * P:(km + 1) * P, :], zf[:])
                    nc.gpsimd.collective_compute("AllReduce", ALU.add, replica_groups=RG,
                                                 ins=[arm_in[nj].opt()],
                                                 outs=[arm_out[nj].opt()])
            return arm_out

        # ---------- FFN halves ----------
        def ffn1_half(l, nj, wp, u, b1_pp):
            c0, c1 = nj * HS, (nj + 1) * HS
            for g in range(2):
                psu = [pp.tile([P, HS], F32, name=f"psu{mi}", tag=f"b{nj * 4 + mi}")
                       for mi in range(4)]
                for ki in range(KT):
                    wk = wp.tile([P, HS], F32R, name="w1_t", tag="w1_t", bufs=4)
                    nc.scalar.dma_start(wk[:], ins[f"w1_{l}"][ki * P:(ki + 1) * P,
                                                              g * HS:(g + 1) * HS])
                    for mi in range(4):
                        nc.tensor.matmul(psu[mi][:], wk[:, mi * P:(mi + 1) * P],
                                         yr(ki, c0, c1),
                                         start=(ki == 0), stop=(ki == KT - 1))
                for mi in range(4):
                    um = g * 4 + mi
                    nc.scalar.activation(u[um][:, c0:c1], psu[mi][:],
                                         AF.Gelu_apprx_tanh, bias=b1_pp[:, um:um + 1])

        def ffn2_half(l, nj, wp, u, b2e_pp, arf_in, arf_out):
            c0, c1 = nj * HS, (nj + 1) * HS
            w2_v = ins[f"w2_{l}"]
            for hg in range(4):
                psf = [pp.tile([P, HS], F32, name=f"psf{mi}", tag=f"b{nj * 4 + mi}")
                       for mi in range(4)]
                for ki in range(UT):
                    wk2 = wp.tile([P, HS], F32R, name="w2_t", tag="w2_t", bufs=4)
                    nc.scalar.dma_start(wk2[:], w2_v[ki * P:(ki + 1) * P,
                                                     hg * HS:(hg + 1) * HS])
                    for mi in range(4):
                        nc.tensor.matmul(psf[mi][:], wk2[:, mi * P:(mi + 1) * P],
                                         u[ki][:, c0:c1],
                                         start=(ki == 0), stop=(ki == UT - 1))
                for mi in range(4):
                    km = hg * 4 + mi
                    zf = wp.tile([P, HS], F32, name="zf2", tag="zf2", bufs=3)
                    nc.scalar.activation(zf[:], psf[mi][:], AF.Identity,
                                         bias=b2e_pp[:, km:km + 1])
                    nc.gpsimd.dma_start(arf_in[nj][km * P:(km + 1) * P, :], zf[:])
            nc.gpsimd.collective_compute("AllReduce", ALU.add, replica_groups=RG,
                                         ins=[arf_in[nj].opt()],
                                         outs=[arf_out[nj].opt()])

        # =============== main flow ===============
        # Program order is chosen so every engine's linear instruction stream
        # stays unblocked: all half-0 work (ln2 -> ffn1 -> ffn2 -> AR) is
        # emitted before any half-1 work, and the next layer's ln1/qkv/v halves
        # are emitted interleaved so they fill the final AR's latency.
        with tc.tile_pool(name="x0p", bufs=1) as x0p:
            x_cat = x0p.tile([P, 2 * S], F32, name="x_cat", tag="x_cat")
            embed_gather(x_cat)
            ln_gather(0, x_cat)
        attn_prep(0)
        attn_qkv_half(0, 0)
        attn_v_half(0, 0)
        attn_qkv_half(0, 1)
        attn_v_half(0, 1)
        gate_block(0)
        for l in range(L):
            attn_core(l)
            arm_out = moe_phase(l)
            arf_in = [dr.tile([D, HS], F32, name=f"arfi{nj}", tag=f"arfi{nj}_{l}")
                      for nj in range(NJ)]
            arf_out = [dr.tile([D, HS], F32, name=f"arfo{nj}", tag=f"arfo{nj}_{l}",
                               addr_space="Shared") for nj in range(NJ)]
            with tc.tile_pool(name=f"ffn_{l}", bufs=1) as wp:
                b1_pp = wp.tile([P, UT], F32, name="b1_pp", tag="b1_pp")
                nc.sync.dma_start(b1_pp[:], ins[f"b1_{l}"][:])
                b2e_pp = wp.tile([P, KT], F32, name="b2e_pp", tag="b2e_pp")
                nc.sync.dma_start(b2e_pp[:], ins[f"b2e_{l}"][:])
                u = [wp.tile([P, S], F32R, name=f"u{mi}", tag=f"u{mi}") for mi in range(UT)]
                ln_local(l, f"ln2_{l}", arm_out[0], 0, with_gate=False)
                ffn1_half(l, 0, wp, u, b1_pp)
                ffn2_half(l, 0, wp, u, b2e_pp, arf_in, arf_out)
                ln_local(l, f"ln2_{l}", arm_out[1], 1, with_gate=False)
                ffn1_half(l, 1, wp, u, b1_pp)
                ffn2_half(l, 1, wp, u, b2e_pp, arf_in, arf_out)

            # ---------- next activation (LN1 of l+1, or raw for final) ------
            if l + 1 < L:
                nc.sync.dma_start(ws_tl[:].rearrange("p (kt e) -> p kt e", e=E),
                                  ins[f"ws_{l + 1}"][:, :].rearrange(
                                      "(kt p) e -> p kt e", p=P))
                ln_local(l + 1, f"ln1_{l + 1}", arf_out[0], 0,
                         with_gate=True, ws_t=ws_tl)
                attn_prep(l + 1)
                attn_qkv_half(l + 1, 0)
                attn_v_half(l + 1, 0)
                ln_local(l + 1, f"ln1_{l + 1}", arf_out[1], 1,
                         with_gate=True, ws_t=ws_tl)
                attn_qkv_half(l + 1, 1)
                attn_v_half(l + 1, 1)
                gate_block(l + 1)
            else:
                for nj in range(NJ):
                    ln_local(l, "xf", arf_out[nj], nj, with_gate=False, normalize=False)

        # =============== final projection (vocab-split, f32r) ===============
        with tc.tile_pool(name="finp", bufs=1) as wp:
            for vj in range(8):
                wks = []
                for ki in range(KT):
                    wk = wp.tile([P, 500], F32R, name="wo_k", tag="wo_k", bufs=KT + 2)
                    nc.sync.dma_start(wk[:], wo[ki * P:(ki + 1) * P, vj * 500:(vj + 1) * 500])
                    wks.append(wk)
                for g in range(2):
                    psums = [pp.tile([P, 500], F32, name=f"po{mi}", tag=f"b{g * 4 + mi}")
                             for mi in range(4)]
                    for ki in range(KT):
                        for mi in range(4):
                            tb = g * 4 + mi
                            nc.tensor.matmul(psums[mi][:], yr(ki, tb * P, (tb + 1) * P),
                                             wks[ki][:], start=(ki == 0), stop=False)
                    for mi in range(4):
                        # bias: ones stationary row x f32r bias moving row
                        nc.tensor.matmul(psums[mi][:], ones_rr[:],
                                         bo_r[:, vj * 500:(vj + 1) * 500],
                                         start=False, stop=True)
                    for mi in range(4):
                        tb = g * 4 + mi
                        lo = wp.tile([P, 500], F32, name="lo", tag="lo", bufs=4)
                        nc.scalar.copy(lo[:], psums[mi][:])
                        nc.scalar.dma_start(out[tb * P:(tb + 1) * P,
                                              vj * 500:(vj + 1) * 500], lo[:])


def kernel(**inputs):
    if "nc" not in _CACHE:
        _CACHE["nc"] = build_nc()
    nc = _CACHE["nc"]
    in_maps = prepare_in_maps(inputs)
    r = run_bass_kernel_spmd(nc, in_maps, core_ids=list(range(NC)), trace=False)
    logits = np.concatenate([r.results[c]["out"] for c in range(NC)], axis=1)
    return logits.reshape(B, S, V).astype(np.float32)
